# revision 49
# baseline (speedup 1.0000x reference)
"""Trainium2 Bass kernel for channel attention (XCA-style) nn.Module.

One image per NeuronCore (batch=8 over 8 cores). Pipeline per image:
  qkv 1x1 conv (matmul) -> 3x3 depthwise conv -> l2norm channel attention
  -> (attn @ v folded with proj 1x1 into a single matmul).

v2.1 structure:
  * Stripes of 32 rows with 1-row halos; padded row stride 132.
  * q,k transposed to pixel-major via BATCHED dma_start_transpose
    (one 1MB xbar call per (stripe, chunk)) -- no PE transposes.
  * PSUM evacuations batched 2 banks per ACT copy.
  * Gram q@k^T accumulated unnormalized in PSUM across stripes.
  * Depthwise: 5 taps as PE diagonal matmuls (tap-outer loops to share
    LDWEIGHTS), 4 taps as DVE mul+add.
  * v's conv+depthwise for stripes 0-1 runs before the attention
    finalize; finalize overlaps with v-work on other stripes.
  * y in bf16, stored via SWDGE casting DMA.
"""

import numpy as np
import ml_dtypes

import concourse.bass as bass
import concourse.tile as tile
from concourse import mybir, bacc
from concourse.bass_utils import run_bass_kernel_spmd

F32 = mybir.dt.float32
BF16 = mybir.dt.bfloat16
AX = mybir.AxisListType
OP = mybir.AluOpType
ACTF = mybir.ActivationFunctionType

C, H, W = 192, 128, 128
HW = H * W
HEADS, CH = 4, 48
RS = 132
TAPS = [(dy, dx) for dy in (-1, 0, 1) for dx in (-1, 0, 1)]
S = 32
NS = H // S
PE_TAPS = [0, 2, 6, 8, 3]
DVE_TAPS = [t for t in range(9) if t not in PE_TAPS]

_cached = {}


def _build_program():
    nc = bacc.Bacc("TRN2", target_bir_lowering=False, debug=False, num_devices=8)

    x_d = nc.dram_tensor("x", [C, HW], F32, kind="ExternalInput").ap()
    w1t_d = nc.dram_tensor("w1t", [C, 576], BF16, kind="ExternalInput").ap()
    dwt_d = nc.dram_tensor("dwt", [128, 5, 18], F32, kind="ExternalInput").ap()
    pwt_d = nc.dram_tensor("pwt", [48, 4, C], BF16, kind="ExternalInput").ap()
    i96_d = nc.dram_tensor("i96", [96, 96], F32, kind="ExternalInput").ap()
    i128_d = nc.dram_tensor("i128", [128, 128], BF16, kind="ExternalInput").ap()
    t4_d = nc.dram_tensor("t4", [1, 4], F32, kind="ExternalInput").ap()
    y_d = nc.dram_tensor("y", [C, HW], F32, kind="ExternalOutput").ap()

    with tile.TileContext(nc) as tc:
        _emit(nc, tc, x_d, w1t_d, dwt_d, pwt_d, i96_d, i128_d, t4_d, y_d)
    nc.finalize()
    return nc


def _emit(nc, tc, x_d, w1t_d, dwt_d, pwt_d, i96_d, i128_d, t4_d, y_d):
    from contextlib import ExitStack

    with ExitStack() as top:
        persist = top.enter_context(tc.tile_pool(name="persist", bufs=1))

        w1t0 = persist.tile([128, 576], BF16)
        w1t1 = persist.tile([64, 576], BF16)
        nc.sync.dma_start(out=w1t0, in_=w1t_d[0:128, :])
        nc.sync.dma_start(out=w1t1, in_=w1t_d[128:192, :])
        dwt = persist.tile([128, 5, 18], F32)
        nc.scalar.dma_start(out=dwt, in_=dwt_d)
        i128 = persist.tile([128, 128], BF16)
        nc.scalar.dma_start(out=i128, in_=i128_d)
        pwt = persist.tile([48, 4, C], BF16)
        nc.scalar.dma_start(out=pwt, in_=pwt_d)
        i96 = persist.tile([96, 96], F32)
        nc.scalar.dma_start(out=i96, in_=i96_d)
        t4s = persist.tile([1, 4], F32)
        nc.scalar.dma_start(out=t4s, in_=t4_d)
        # preload the Sqrt ACT table set during the idle boot window so the
        # finalize's sqrt doesn't pay the ~2.7us table switch mid-kernel
        sqwarm = persist.tile([1, 4], F32)
        nc.scalar.activation(sqwarm, t4s, ACTF.Sqrt)

        npe = len(PE_TAPS)
        diag = persist.tile([128, 5 * npe, 128], BF16)
        for ci in range(5):
            for j, t in enumerate(PE_TAPS):
                nc.vector.tensor_scalar_mul(diag[:, ci * npe + j, :], i128,
                                            dwt[:, ci, t:t + 1])

        xb0 = persist.tile([128, HW], BF16)
        xb1 = persist.tile([64, HW], BF16)

        mta = persist.tile([128, C], BF16)
        mtb = persist.tile([64, C], BF16)

        pre_p = top.enter_context(tc.tile_pool(name="pre_p", bufs=3))
        tmp_p = top.enter_context(tc.tile_pool(name="tmp_p", bufs=2))
        fpool = top.enter_context(tc.tile_pool(name="fpool", bufs=1))

        ps_sh = top.enter_context(tc.tile_pool(name="ps_sh", bufs=1,
                                               space="PSUM"))
        cp_sh = top.enter_context(tc.tile_pool(name="cp_sh", bufs=1,
                                               space="PSUM"))
        gpool = top.enter_context(tc.tile_pool(name="gpool", bufs=1,
                                               space="PSUM"))
        g_ps = gpool.tile([96, HEADS, 96], F32)
        yq_p = top.enter_context(tc.tile_pool(name="yq_p", bufs=3,
                                              space="PSUM"))

        def load_stripe(s):
            px = slice(s * S * W, (s + 1) * S * W)
            nc.gpsimd.dma_start(out=xb0[:, px], in_=x_d[0:128, px])
            nc.gpsimd.dma_start(out=xb1[:, px], in_=x_d[128:192, px])

        # HAM warm-up: keep the PE busy during the initial load window so
        # the clock is at 2.4GHz when the first real matmuls arrive.
        warm = ps_sh.tile([128, 2, 512], F32, tag="mmps", name="mmps")
        for r in range(50):
            nc.tensor.matmul(warm[:, r % 2, 0:128], i128, i128,
                             start=True, stop=True)
        wsink = persist.tile([128, 128], F32)
        nc.vector.tensor_copy(wsink, warm[:, 0, 0:128])

        # ================= pass A: q,k =================
        with ExitStack() as pa:
            acc_p = pa.enter_context(tc.tile_pool(name="acc_p", bufs=3))
            qkt_p = pa.enter_context(tc.tile_pool(name="qkt_p", bufs=2))
            for s in range(NS + 1):
                if s < NS:
                    load_stripe(s)
                if s >= 1:
                    _conv_stripe_qk(nc, s - 1, w1t0, w1t1, xb0, xb1,
                                    dwt, diag, pre_p, tmp_p, acc_p, qkt_p,
                                    ps_sh, cp_sh, g_ps)

        # ================= pass B =================
        if True:
            accv0_p = top.enter_context(tc.tile_pool(name="accv0_p", bufs=4))
            accv1_p = top.enter_context(tc.tile_pool(name="accv1_p", bufs=4))
            y_p = top.enter_context(tc.tile_pool(name="y_p", bufs=3))
            accv = {}
            def vdw(s):
                accv[s] = _v_pre_dw(nc, s, w1t0, w1t1, xb0, xb1, dwt, diag,
                                    pre_p, tmp_p, accv0_p, accv1_p,
                                    ps_sh, cp_sh)

            vdw(0)
            vdw(1)
            vdw(2)

            # ============ finalize attention -> MT ============
            with ExitStack() as fz:
                fps_t64 = yq_p.tile([64, 512], F32, tag="yp1", name="yp1")
                fps_t = fps_t64[0:48, :]
                gs = fpool.tile([96, HEADS, 96], F32)
                nc.scalar.copy(gs, g_ps)

                i96b = bass.AP(tensor=i96.tensor, offset=i96.offset,
                               ap=[list(i96.ap[0]), [0, HEADS], [1, 96]])
                gdiag = fpool.tile([96, HEADS, 96], F32)
                nc.vector.tensor_mul(gdiag, gs, i96b)
                nrm2 = fpool.tile([96, HEADS], F32)
                nc.vector.reduce_sum(nrm2, gdiag, axis=AX.X)
                nrm = fpool.tile([96, HEADS], F32)
                nc.scalar.activation(nrm, nrm2, ACTF.Sqrt)
                nc.vector.tensor_scalar_max(nrm, nrm, 1e-12)
                rstd = fpool.tile([96, HEADS], F32)
                nc.vector.reciprocal(rstd, nrm)

                t4b = fpool.tile([48, HEADS], F32)
                nc.sync.dma_start(
                    out=t4b,
                    in_=bass.AP(tensor=t4_d.tensor, offset=t4_d.offset,
                                ap=[[0, 48], [1, HEADS]]))
                rq = fpool.tile([48, HEADS], F32)
                nc.vector.tensor_mul(rq, rstd[0:48, :], t4b)

                rkk = fpool.tile([48, HEADS], F32)
                nc.sync.dma_start(out=rkk, in_=rstd[48:96, :])
                rkps = fps_t[0:4, 0:48]
                nc.tensor.transpose(rkps, rkk, i96[0:48, 0:48])
                rkrow = fpool.tile([4, 48], F32)
                nc.vector.tensor_copy(rkrow, rkps)
                dram_p = fz.enter_context(tc.tile_pool(name="dram_p", bufs=1,
                                                       space="DRAM"))
                rkd = dram_p.tile([4, 48], F32)
                nc.sync.dma_start(out=rkd, in_=rkrow)
                rk = fpool.tile([48, HEADS, 48], F32)
                for h in range(HEADS):
                    bsrc = bass.AP(tensor=rkd.tensor,
                                   offset=rkd.offset + h * 48,
                                   ap=[[0, 48], [1, 48]])
                    nc.sync.dma_start(out=rk[:, h, :], in_=bsrc)

                z = fpool.tile([48, HEADS, 48], F32)
                for h in range(HEADS):
                    nc.vector.scalar_tensor_tensor(
                        out=z[:, h, :], in0=gs[0:48, h, 48:96],
                        scalar=rq[:, h:h + 1], in1=rk[:, h, :],
                        op0=OP.mult, op1=OP.mult)
                mx = fpool.tile([48, HEADS], F32)
                nc.vector.reduce_max(mx, z, axis=AX.X)
                nmx = fpool.tile([48, HEADS], F32)
                nc.vector.tensor_scalar_mul(nmx, mx, -1.0)
                ez = fpool.tile([48, HEADS, 48], F32)
                for h in range(HEADS):
                    nc.scalar.activation(ez[:, h, :], z[:, h, :], ACTF.Exp,
                                         bias=nmx[:, h:h + 1], scale=1.0)
                sm = fpool.tile([48, HEADS], F32)
                nc.vector.reduce_sum(sm, ez, axis=AX.X)
                rs = fpool.tile([48, HEADS], F32)
                nc.vector.reciprocal(rs, sm)
                a_bf = fpool.tile([48, HEADS, 48], BF16)
                for h in range(HEADS):
                    nc.vector.tensor_scalar_mul(a_bf[:, h, :], ez[:, h, :],
                                                rs[:, h:h + 1])

                m_bf = fpool.tile([48, HEADS, C], BF16)
                for h in range(HEADS):
                    mps = fps_t[0:48, 256:448]
                    nc.tensor.matmul(mps, a_bf[:, h, :], pwt[:, h, :],
                                     start=True, stop=True)
                    nc.scalar.copy(m_bf[:, h, :], mps)

                nc.sync.dma_start(out=mta[0:48, :], in_=m_bf[:, 0, :])
                nc.sync.dma_start(out=mta[48:96, :], in_=m_bf[:, 1, :])
                nc.sync.dma_start(out=mta[96:128, :], in_=m_bf[0:32, 2, :])
                nc.sync.dma_start(out=mtb[0:16, :], in_=m_bf[32:48, 2, :])
                nc.sync.dma_start(out=mtb[16:64, :], in_=m_bf[:, 3, :])

            # ============ outputs ============
            _v_out(nc, 0, accv[0], mta, mtb, ps_sh, yq_p, y_p, y_d)
            vdw(3)
            _v_out(nc, 1, accv[1], mta, mtb, ps_sh, yq_p, y_p, y_d)
            _v_out(nc, 2, accv[2], mta, mtb, ps_sh, yq_p, y_p, y_d)
            _v_out(nc, 3, accv[3], mta, mtb, ps_sh, yq_p, y_p, y_d)


def _stripe_matmul(nc, s, oc_defs, w1t0, w1t1, xb0, xb1, pre, ps_p, pre_bufs):
    """1x1 conv into padded [p, S+2, RS] stripe tiles (halo rows included).
    cc-outer matmul order shares LDWEIGHTS across the 2 banks of a batch."""
    r0 = s * S
    lo, hi = max(r0 - 1, 0), min(r0 + S + 1, H)
    for i, (ocp, ocsl) in enumerate(oc_defs):
        p = pre[i]
        if s < pre_bufs:
            nc.gpsimd.memset(p[:, :, 0:2], 0.0)
            nc.gpsimd.memset(p[:, :, 130:132], 0.0)
        if s == 0:
            nc.gpsimd.memset(p[:, 0, :], 0.0)
        if s == NS - 1:
            nc.gpsimd.memset(p[:, S + 1, :], 0.0)
        r = lo
        while r < hi:
            nr = min(8, hi - r)
            ps = ps_p.tile([ocp, 2, 512], F32, tag="mmps", name="mmps")
            spans = []
            rr = r
            for j2 in range(2):
                if rr >= hi:
                    break
                nj = min(4, hi - rr)
                spans.append((j2, slice(rr * W, (rr + nj) * W), nj))
                rr += nj
            for cc, (wt, xbt) in enumerate(((w1t0, xb0), (w1t1, xb1))):
                for (j2, px, nj) in spans:
                    nc.tensor.matmul(ps[:, j2, 0:nj * W], wt[:, ocsl],
                                     xbt[:, px], start=(cc == 0),
                                     stop=(cc == 1))
            tr = r - (r0 - 1)
            fl = ps.rearrange("p a b -> p (a b)")
            nc.scalar.copy(
                p[:, tr:tr + nr, 2:130],
                fl[:, 0:nr * W].rearrange("p (a b) -> p a b", b=W))
            r += nr


def _dw_conv(nc, pre, tmp_p, acc, dwt, diag, oc_list, cps_p):
    """3x3 depthwise conv on padded [p, S+2, RS] tiles -> acc [p, S, W].
    PE_TAPS as diagonal matmuls, tap-outer so LDWEIGHTS is shared across
    the 2 banks; ACT evacuates as accumulator init; DVE taps mul+add."""
    npe = len(PE_TAPS)
    for i, oc in enumerate(oc_list):
        p, a = pre[i], acc[i]
        np_ = p.shape[0]
        for g2 in range(S // 8):
            cp = cps_p.tile([np_, 2, 512], F32, tag="cps", name="cps")
            for j, t in enumerate(PE_TAPS):
                dy, dx = TAPS[t]
                for j2 in range(2):
                    g = 2 * g2 + j2
                    rhs = p[:, 1 + dy + 4 * g: 1 + dy + 4 * g + 4,
                            2 + dx: 130 + dx]
                    nc.tensor.matmul(cp[:, j2, :],
                                     diag[:np_, oc * npe + j, :np_],
                                     rhs, start=(j == 0), stop=(j == npe - 1))
            nc.scalar.copy(a[:, 8 * g2:8 * g2 + 8, :],
                           cp.rearrange("p a b -> p (a b)").rearrange(
                               "p (a b) -> p a b", b=W))
        for t in DVE_TAPS:
            dy, dx = TAPS[t]
            wv = dwt[:np_, oc, t:t + 1]
            v = p[:, 1 + dy: 1 + dy + S, 2 + dx: 130 + dx]
            tm = tmp_p.tile([128, S, W], BF16, tag="tmp", name="tmp")
            nc.vector.tensor_scalar_mul(tm[:np_], v, wv)
            nc.vector.tensor_add(a, a, tm[:np_])


def _conv_stripe_qk(nc, s, w1t0, w1t1, xb0, xb1, dwt, diag,
                    pre_p, tmp_p, acc_p, qkt_p, ps_p, cps_p, g_ps):
    pre = [pre_p.tile([128, S + 2, RS], BF16, tag="pre", name=f"pre{i}")
           for i in range(3)]
    oc_defs = [(128, slice(0, 128)), (128, slice(128, 256)),
               (128, slice(256, 384))]
    _stripe_matmul(nc, s, oc_defs, w1t0, w1t1, xb0, xb1, pre, ps_p,
                   pre_bufs=3)

    acc = [acc_p.tile([128, S, W], BF16, tag="acc", name=f"acc{i}")
           for i in range(3)]
    _dw_conv(nc, pre, tmp_p, acc, dwt, diag, [0, 1, 2], cps_p)

    qkt = qkt_p.tile([128, S, 384], BF16, tag="qkt", name="qkt")
    for i in range(3):
        nc.sync.dma_start_transpose(qkt[:, :, 128 * i:128 * (i + 1)], acc[i])

    first = (s == 0)
    last = (s == NS - 1)
    for pc in range(S):
        for h in range(HEADS):
            nc.tensor.matmul(
                g_ps[:, h, :], qkt[:, pc, 96 * h:96 * h + 96],
                qkt[:, pc, 96 * h:96 * h + 96],
                start=(first and pc == 0),
                stop=(last and pc == S - 1),
                skip_group_check=True)


def _v_pre_dw(nc, s, w1t0, w1t1, xb0, xb1, dwt, diag,
              pre_p, tmp_p, accv0_p, accv1_p, ps_p, cps_p):
    pre = [pre_p.tile([128, S + 2, RS], BF16, tag="pre", name="prev0"),
           pre_p.tile([64, S + 2, RS], BF16, tag="pre", name="prev1")]
    oc_defs = [(128, slice(384, 512)), (64, slice(512, 576))]
    _stripe_matmul(nc, s, oc_defs, w1t0, w1t1, xb0, xb1, pre, ps_p,
                   pre_bufs=3)

    acc = [accv0_p.tile([128, S, W], BF16, tag="accv0", name="accv0"),
           accv1_p.tile([64, S, W], BF16, tag="accv1", name="accv1")]
    _dw_conv(nc, pre, tmp_p, acc, dwt, diag, [3, 4], cps_p)
    return acc


def _v_out(nc, s, acc, mta, mtb, ps_p, yq_p, y_p, y_d):
    """y = M^T.T @ v; stationary-outer matmul order; yp1 evac on DVE."""
    r0 = s * S
    for g2 in range(S // 8):
        pxs = [slice(8 * g2, 8 * g2 + 4), slice(8 * g2 + 4, 8 * g2 + 8)]
        dpx = slice(r0 * W + 1024 * g2, r0 * W + 1024 * (g2 + 1))
        yp0 = ps_p.tile([128, 2, 512], F32, tag="mmps", name="mmps")
        for (st, src, first, lastf) in (
                (mta[:, 0:128], acc[0], True, False),
                (mtb[:, 0:128], acc[1], False, True)):
            for j2 in range(2):
                nc.tensor.matmul(yp0[:, j2, :], st, src[:, pxs[j2], :],
                                 start=first, stop=lastf)
        y0 = y_p.tile([128, 1024], BF16, tag="y0", name="y0")
        nc.scalar.copy(y0, yp0.rearrange("p a b -> p (a b)"))
        nc.gpsimd.dma_start(out=y_d[0:128, dpx], in_=y0)
        y1 = y_p.tile([64, 1024], BF16, tag="y1", name="y1")
        for j2 in range(2):
            yp1 = yq_p.tile([64, 512], F32, tag="yp1", name="yp1")
            nc.tensor.matmul(yp1, mta[:, 128:192], acc[0][:, pxs[j2], :],
                             start=True, stop=False)
            nc.tensor.matmul(yp1, mtb[:, 128:192], acc[1][:, pxs[j2], :],
                             start=False, stop=True)
            if j2 == 0:
                nc.vector.tensor_copy(y1[:, 0:512], yp1)
            else:
                nc.scalar.copy(y1[:, 512:1024], yp1)
        nc.gpsimd.dma_start(out=y_d[128:192, dpx], in_=y1)


# ---------------- host glue ----------------

def _host_inputs(x, qkv_w, dw_w, proj_w, temperature):
    perm = []
    for h in range(HEADS):
        perm += list(range(h * CH, (h + 1) * CH))
        perm += list(range(C + h * CH, C + (h + 1) * CH))
    perm += list(range(2 * C, 3 * C))
    perm = np.array(perm)

    w1 = np.asarray(qkv_w)[perm]
    w1t = np.ascontiguousarray(w1.T).astype(ml_dtypes.bfloat16)
    dw = np.asarray(dw_w)[perm, 0]
    dwt = np.zeros((128, 5, 18), np.float32)
    for ci in range(5):
        rows = min(128, 576 - ci * 128)
        taps = dw[ci * 128: ci * 128 + rows].reshape(rows, 9)
        dwt[:rows, ci, 0:9] = taps
        dwt[:rows, ci, 9:18] = -taps
    pT = np.asarray(proj_w).T.astype(np.float32)
    pwt = np.stack([pT[48 * h:48 * (h + 1)] for h in range(4)],
                   axis=1).astype(ml_dtypes.bfloat16)
    i96 = np.eye(96, dtype=np.float32)
    i128 = np.eye(128, dtype=ml_dtypes.bfloat16)
    t4 = np.asarray(temperature).reshape(1, HEADS).astype(np.float32)
    shared = {
        "w1t": w1t, "dwt": dwt, "pwt": pwt, "i96": i96, "i128": i128,
        "t4": t4,
    }
    xs = np.asarray(x).reshape(8, C, HW).astype(np.float32)
    return shared, xs


def kernel(x, qkv_w, dw_w, proj_w, temperature, _trace=False):
    if "nc" not in _cached:
        _cached["nc"] = _build_program()
    nc = _cached["nc"]
    shared, xs = _host_inputs(x, qkv_w, dw_w, proj_w, temperature)
    in_maps = [dict(shared, x=np.ascontiguousarray(xs[i])) for i in range(8)]
    res = run_bass_kernel_spmd(nc, in_maps, core_ids=list(range(8)),
                               trace=_trace)
    out = np.stack([np.asarray(res.results[i]["y"]).reshape(C, H, W)
                    for i in range(8)])
    if _trace:
        _cached["last_exec_time_ns"] = res.exec_time_ns
        _cached["last_results"] = res
    return out
